# revision 9
# baseline (speedup 1.0000x reference)
"""Single-head causal attention (B=4, T=4096, C=1024, H=128) on 8 NeuronCores.

Sharding: 2 cores per batch, interleaved 512-row q-blocks. Host permutes each
core's xT columns (role-1 swaps the 512-col halves of every 1024-col group) so
every core's q-blocks live at the SAME static offsets: q-block m = program
cols [1024m, 1024m+512). kv tiles are processed in program (permuted) order;
causal masks become static for the 4-tile diagonal window plus a per-core 0/1
scalar for the other half-group — identical SPMD program, role as data.

Device program (per core, matmuls bf16, f32 PSUM):
  Q^T block m = WqT.T @ x[:, 1024m:1024m+512]     (Wq pre-scaled by 1/sqrt(H))
  K^T / V^T per 512-col tile group; V^T -> V via DMA xbar transpose (no PE)
  attention per q-block (512 q), flash-style without running max:
    S^T[kv,q] = K_j^T.T @ Q^T ; P = exp(S^T) * mask ; U += P (DVE/GpSimd)
    outT[h,q] += V_j.T @ P
  epilogue: denom row = ones.T @ (U0;U1) on PE -> reciprocal -> rank-1
    broadcast MM -> outT * rec (DVE) -> DMA out in [H, q] layout (host
    transposes back). K/V/Q projection MMs are interleaved into the
    ACT(exp)-bound attention stream via a deadline-tagged op queue.
"""
import os
import sys

import numpy as np

try:
    import ml_dtypes
except ImportError:  # pragma: no cover
    sys.path.insert(0, "/opt/trn_rl_repo")
    import ml_dtypes

for _p in ("/opt/trn_rl_repo",):
    if os.path.isdir(_p) and _p not in sys.path:
        sys.path.insert(0, _p)

try:
    import jax as _jax
    _jax.config.update("jax_compilation_cache_dir", "/tmp/jax_neff_cache")
    _jax.config.update("jax_persistent_cache_min_entry_size_bytes", -1)
    _jax.config.update("jax_persistent_cache_min_compile_time_secs", 0.0)
except Exception:
    pass

import concourse.bass as bass
import concourse.mybir as mybir
import concourse.tile as tile
from concourse import bacc
from concourse.bass_utils import run_bass_kernel_spmd

B, T, C, H = 4, 4096, 1024, 128
P = 128           # partitions / tile edge
CK = C // P       # 8 contraction chunks
QW = 512          # q-block width
NQB = 4           # q-blocks per core (2048 queries)
KV_TILES = T // P # 32
NQ = NQB * QW
BF16 = ml_dtypes.bfloat16
SCALE = float(np.sqrt(H))

_prog_cache = {}


def _build_program(loop_n=None, loads_in_loop=True) -> bass.Bass:
    nc = bacc.Bacc("TRN2")
    dt = mybir.dt

    xT_d = nc.declare_dram_parameter("xT", [C, T], dt.bfloat16, isOutput=False)
    w_d = nc.declare_dram_parameter("w_all", [C, 3 * H], dt.bfloat16, isOutput=False)
    masks_d = nc.declare_dram_parameter("masks", [P, 4 * QW], dt.bfloat16, isOutput=False)
    rs_d = nc.declare_dram_parameter("rs", [P, 1], dt.float32, isOutput=False)
    out_d = nc.declare_dram_parameter("out", [H, NQ], dt.float32, isOutput=True)

    with tile.TileContext(nc) as tc:
        with (
            tc.tile_pool(name="consts", bufs=1) as consts,
            tc.tile_pool(name="bigx", bufs=1) as bigx,
            tc.tile_pool(name="persist", bufs=1) as persist,
            tc.tile_pool(name="psum_proj", bufs=2, space="PSUM") as psum_proj,
            tc.tile_pool(name="psum_s", bufs=2, space="PSUM") as psum_s,
            tc.tile_pool(name="psum_o", bufs=1, space="PSUM") as psum_o,
            tc.tile_pool(name="psum_epi", bufs=1, space="PSUM") as psum_epi,
            tc.tile_pool(name="sb_p", bufs=4) as sb_p,
            tc.tile_pool(name="sb_u", bufs=2) as sb_u,
            tc.tile_pool(name="sb_vt", bufs=2) as sb_vt,
            tc.tile_pool(name="sb_o", bufs=2) as sb_o,
        ):
            f32, bf16 = dt.float32, dt.bfloat16
            import contextlib

            def loop_or_null(active):
                return tc.For_i(0, loop_n, 1) if (loop_n and active) else contextlib.nullcontext()

            with loop_or_null(loads_in_loop):

                # ---- constants ----
                w_sb = consts.tile([P, CK * 3 * H], bf16, tag="w")
                masks_sb = consts.tile([P, 4 * QW], bf16, tag="masks")
                rs_sb = consts.tile([P, 1], f32, tag="rs")
                ones_col = consts.tile([P, 1], f32, tag="onesc")
                nc.gpsimd.memset(ones_col[:], 1.0)
                ones_row = consts.tile([1, P], bf16, tag="onesr")
                nc.gpsimd.memset(ones_row[:], 1.0)

                def wq_s(ck):
                    return w_sb[:, ck * 3 * H: ck * 3 * H + H]

                def wk_s(ck):
                    return w_sb[:, ck * 3 * H + H: ck * 3 * H + 2 * H]

                def wv_s(ck):
                    return w_sb[:, ck * 3 * H + 2 * H: ck * 3 * H + 3 * H]

                # ---- stream inputs on the ACT hwdge queue (issue order =
                # consumption order); SP queue is reserved for the V xbar
                # transposes + output stores so they aren't head-of-line
                # blocked behind multi-MB input chunks ----
                x_sb = bigx.tile([P, CK * T], bf16, tag="x")
                x3 = x_sb[:].rearrange("p (ck t) -> p ck t", t=T)
                xd3 = xT_d.ap().rearrange("(ck p) t -> p ck t", p=P)

                nc.scalar.dma_start(
                    w_sb[:].rearrange("p (ck h) -> p ck h", h=3 * H),
                    w_d.ap().rearrange("(ck p) h -> p ck h", p=P),
                )
                nc.scalar.dma_start(x3[:, :, 0:QW], xd3[:, :, 0:QW])
                nc.scalar.dma_start(masks_sb[:], masks_d.ap()[:])
                nc.scalar.dma_start(rs_sb[:], rs_d.ap()[:])
                for hseg in range(1, 8):
                    nc.scalar.dma_start(
                        x3[:, :, hseg * QW:(hseg + 1) * QW],
                        xd3[:, :, hseg * QW:(hseg + 1) * QW],
                    )

                kT_sb = persist.tile([P, T], bf16, tag="kT")
                v_sb = persist.tile([P, KV_TILES * H], bf16, tag="v")
                qT_sb = persist.tile([P, NQ], bf16, tag="qT")

                with loop_or_null(not loads_in_loop):
                    # ---- interleaved-op queue: (deadline_tile, x_seg, closure).
                    # Closures are popped either at their deadline (flush: last
                    # moment before the first attention trip that consumes their
                    # output — correctness) or at their statically scheduled
                    # attention pair (arrival-aware: never queue a PE op whose
                    # x segment hasn't landed, it would head-of-line block the
                    # in-order PE queue). ----
                    pending = []

                    def emit_qproj(m):
                        # q-block m reads program cols [1024m, 1024m+512)
                        st = {}

                        def mk(ck):
                            def op():
                                if ck == 0:
                                    st["ps"] = psum_proj.tile([P, QW], f32, tag="proj", name="psq")
                                nc.tensor.matmul(
                                    st["ps"][:],
                                    lhsT=wq_s(ck),
                                    rhs=x_sb[:, ck * T + 2 * m * QW: ck * T + 2 * m * QW + QW],
                                    start=(ck == 0), stop=(ck == CK - 1),
                                )
                            return op

                        tag, seg = 8 * m, 2 * m
                        for ck in range(CK):
                            pending.append((tag, seg, mk(ck)))
                        pending.append((tag, seg, lambda: nc.vector.tensor_scalar_mul(
                            qT_sb[:, m * QW:(m + 1) * QW], st["ps"][:], 1.0)))

                    def emit_projtile(t):
                        # K^T and V for kv tiles 4t..4t+3 (program cols 512t..+512)
                        tag, seg = 4 * t, t
                        stk, stv = {}, {}

                        def mkk(ck):
                            def op():
                                if ck == 0:
                                    stk["ps"] = psum_proj.tile([P, QW], f32, tag="proj", name="psk")
                                nc.tensor.matmul(
                                    stk["ps"][:],
                                    lhsT=wk_s(ck),
                                    rhs=x_sb[:, ck * T + t * QW: ck * T + (t + 1) * QW],
                                    start=(ck == 0), stop=(ck == CK - 1),
                                )
                            return op

                        def mkv(ck):
                            def op():
                                if ck == 0:
                                    stv["ps"] = psum_proj.tile([P, QW], f32, tag="proj", name="psv")
                                nc.tensor.matmul(
                                    stv["ps"][:],
                                    lhsT=wv_s(ck),
                                    rhs=x_sb[:, ck * T + t * QW: ck * T + (t + 1) * QW],
                                    start=(ck == 0), stop=(ck == CK - 1),
                                )
                            return op

                        for ck in range(CK):
                            pending.append((tag, seg, mkk(ck)))
                        pending.append((tag, seg, lambda: nc.vector.tensor_scalar_mul(
                            kT_sb[:, t * QW:(t + 1) * QW], stk["ps"][:], 1.0)))
                        for ck in range(CK):
                            pending.append((tag, seg, mkv(ck)))

                        def vcopy():
                            stv["vt"] = sb_vt.tile([P, QW], bf16, tag="vt", name="vt")
                            nc.vector.tensor_scalar_mul(stv["vt"][:], stv["ps"][:], 1.0)
                        pending.append((tag, seg, vcopy))

                        def mktr(s):
                            def op():
                                nc.sync.dma_start_transpose(
                                    v_sb[:, (4 * t + s) * H:(4 * t + s + 1) * H],
                                    stv["vt"][:, s * P:(s + 1) * P],
                                )
                            return op

                        for s in range(4):
                            pending.append((4 * t + s, seg, mktr(s)))

                    # queue everything in deadline order
                    for m in range(NQB):
                        emit_qproj(m)
                        emit_projtile(2 * m)
                        emit_projtile(2 * m + 1)

                    # ---- static clock model: estimated start time (us) of each
                    # attention pair, assuming ~0.358 GB/us HBM and ACT-bound
                    # pairs of ~1.15us. Input order: w(0.75MB), seg0(1MB),
                    # masks(0.5MB), rs, seg1..7 (1MB each). ----
                    def arrive(seg):
                        mb = 0.75 + 1.0 + (0.53 + seg * 1.0 if seg >= 1 else 0.0)
                        return mb / 0.358

                    pair_start = {}
                    t_est = 0.0
                    for m in range(NQB):
                        for g in range(4 * (m + 1)):
                            t_est = max(
                                t_est,
                                arrive((2 * g + 1) // 4) + 0.4,
                                arrive(2 * m) + 0.3,
                            )
                            pair_start[(m, g)] = t_est
                            t_est += 1.15
                    # assign each closure to the first pair whose start covers
                    # its x segment arrival, capped at 6 closures per pair
                    all_pairs = sorted(pair_start, key=lambda k: pair_start[k])
                    cap = {p: 6 for p in all_pairs}
                    assign = {}
                    for idx, (tag, seg, _) in enumerate(pending):
                        tgt = None
                        for pkey in all_pairs:
                            if pair_start[pkey] >= arrive(seg) + 0.3 and cap[pkey] > 0:
                                tgt = pkey
                                break
                        if tgt is not None:
                            cap[tgt] -= 1
                            assign[idx] = tgt
                    pend_idx = list(range(len(pending)))

                    def pop_assigned(pkey):
                        morder = {k: i for i, k in enumerate(all_pairs)}
                        while pending:
                            idx = pend_idx[0]
                            tgt = assign.get(idx)
                            if tgt is not None and morder[tgt] <= morder[pkey]:
                                pend_idx.pop(0)
                                pending.pop(0)[2]()
                            else:
                                break

                    def flush(tile_id):
                        while pending and pending[0][0] <= tile_id:
                            pend_idx.pop(0)
                            pending.pop(0)[2]()

                    # ---- attention blocks ----
                    for m in range(NQB):
                        trips = 8 * (m + 1)
                        flush(8 * m)  # qT block m + early kv tiles
                        po = psum_o.tile([P, QW], f32, tag="po")
                        U0 = sb_u.tile([P, QW], f32, tag="U0")
                        U1 = sb_u.tile([P, QW], f32, tag="U1")
                        qs = qT_sb[:, m * QW:(m + 1) * QW]
                        for g in range(trips // 2):
                            flush(2 * g + 1)
                            s2 = psum_s.tile([P, 2 * QW], f32, tag="s")
                            for u in range(2):
                                j = 2 * g + u
                                nc.tensor.matmul(
                                    s2[:, u * QW:(u + 1) * QW],
                                    lhsT=kT_sb[:, j * P:(j + 1) * P],
                                    rhs=qs,
                                    start=True, stop=True,
                                )
                            p2 = sb_p.tile([P, 2 * QW], bf16, tag="p")
                            nc.scalar.activation(
                                p2[:], s2[:], mybir.ActivationFunctionType.Exp
                            )
                            pop_assigned((m, g))
                            for u in range(2):
                                j = 2 * g + u
                                pj = p2[:, u * QW:(u + 1) * QW]
                                d = j - 8 * m
                                if 0 <= d < 4:
                                    nc.vector.tensor_mul(
                                        pj, pj, masks_sb[:, d * QW:(d + 1) * QW]
                                    )
                                elif 4 <= d < 8:
                                    nc.vector.tensor_scalar_mul(pj, pj, rs_sb[:, 0:1])
                                ueng = nc.vector if u == 0 else nc.gpsimd
                                Ux = U0 if u == 0 else U1
                                if j < 2:
                                    ueng.tensor_copy(Ux[:], pj)
                                else:
                                    ueng.tensor_add(Ux[:], Ux[:], pj)
                                nc.tensor.matmul(
                                    po[:],
                                    lhsT=v_sb[:, j * H:(j + 1) * H],
                                    rhs=pj,
                                    start=(j == 0), stop=(j == trips - 1),
                                )
                        # epilogue: denom row, reciprocal, rank-1 broadcast, scale
                        epi = psum_epi.tile([P, QW], f32, tag="epi")
                        nc.tensor.matmul(
                            epi[0:1, :], lhsT=ones_col[:], rhs=U0[:],
                            start=True, stop=False,
                        )
                        nc.tensor.matmul(
                            epi[0:1, :], lhsT=ones_col[:], rhs=U1[:],
                            start=False, stop=True,
                        )
                        rec_row = sb_o.tile([1, QW], bf16, tag="rrow")
                        with nc.allow_low_precision(reason="bf16 softmax denom reciprocal; ~2^-9 rel, tol 2e-2"):
                            nc.vector.reciprocal(rec_row[:], epi[0:1, :])
                        nc.tensor.matmul(
                            epi[:], lhsT=ones_row[:], rhs=rec_row[:],
                            start=True, stop=True,
                        )
                        rec_b = sb_o.tile([P, QW], f32, tag="rb")
                        nc.vector.tensor_scalar_mul(rec_b[:], epi[:], 1.0)
                        o = sb_o.tile([P, QW], f32, tag="o")
                        nc.vector.tensor_mul(o[:], po[:], rec_b[:])
                        nc.sync.dma_start(
                            out_d.ap()[:, m * QW:(m + 1) * QW], o[:]
                        )
                    flush(KV_TILES)
    nc.compile()
    return nc


def _make_core_inputs(x, Wq, Wk, Wv):
    w_all = np.concatenate([Wq.T / SCALE, Wk.T, Wv.T], axis=1)  # [C, 3H]
    w_all = np.ascontiguousarray(w_all).astype(BF16)
    # static diagonal-window masks [P, 4*QW]: program tile d vs q-subtile s:
    # d == s -> tri (kv row p attends q col c iff p <= c); d > s -> 0; d < s -> 1
    tri = np.triu(np.ones((P, P), np.float32))
    masks = np.zeros((4, 4, P, P), np.float32)  # [d, s, p, c]
    for d in range(4):
        for s in range(4):
            if d == s:
                masks[d, s] = tri
            elif d < s:
                masks[d, s] = 1.0
    masks_flat = np.ascontiguousarray(
        masks.transpose(2, 0, 1, 3).reshape(P, 4 * QW)).astype(BF16)
    in_maps, qrows_all = [], []
    for c in range(8):
        b, r = c // 2, c % 2
        xT = np.ascontiguousarray(x[b].T)  # [C, T]
        if r:
            xT = np.ascontiguousarray(
                xT.reshape(C, 4, 2, QW)[:, :, ::-1, :].reshape(C, T))
        qrows = np.concatenate(
            [np.arange(QW * (r + 2 * m), QW * (r + 2 * m) + QW) for m in range(NQB)]
        )
        in_maps.append(dict(
            xT=xT.astype(BF16),
            w_all=w_all,
            masks=masks_flat,
            rs=np.full((P, 1), float(r), np.float32),
        ))
        qrows_all.append(qrows)
    return in_maps, qrows_all


def kernel(x, Wq, Wk, Wv):
    x = np.asarray(x, dtype=np.float32)
    if "nc" not in _prog_cache:
        _prog_cache["nc"] = _build_program()
    nc = _prog_cache["nc"]
    in_maps, qrows_all = _make_core_inputs(
        x, np.asarray(Wq, np.float32), np.asarray(Wk, np.float32),
        np.asarray(Wv, np.float32)
    )
    res = run_bass_kernel_spmd(nc, in_maps, list(range(8))).results
    full = np.zeros((B, T, H), np.float32)
    for c in range(8):
        full[c // 2][qrows_all[c]] = res[c]["out"].T
    return full


if __name__ == "__main__":
    nc = _build_program()
    print("program built ok")


# revision 11
# speedup vs baseline: 1.1111x; 1.1111x over previous
"""Single-head causal attention (B=4, T=4096, C=1024, H=128) on 8 NeuronCores.

Sharding: 2 cores per batch, interleaved 512-row q-blocks. Host permutes each
core's xT columns (role-1 swaps the 512-col halves of every 1024-col group) so
every core's q-blocks live at the SAME static offsets: q-block m = program
cols [1024m, 1024m+512). kv tiles are processed in program (permuted) order;
causal masks become static for the 4-tile diagonal window plus a per-core 0/1
scalar for the other half-group — identical SPMD program, role as data.

Device program (per core, matmuls bf16, f32 PSUM):
  Q^T block m = WqT.T @ x[:, 1024m:1024m+512]     (Wq pre-scaled by 1/sqrt(H))
  K^T / V^T per 512-col tile group; V^T -> V via DMA xbar transpose (no PE)
  attention per q-block (512 q), flash-style without running max:
    S^T[kv,q] = K_j^T.T @ Q^T ; P = exp(S^T) * mask ; U += P (DVE/GpSimd)
    outT[h,q] += V_j.T @ P
  epilogue: denom row = ones.T @ (U0;U1) on PE -> reciprocal -> rank-1
    broadcast MM -> outT * rec (DVE) -> DMA out in [H, q] layout (host
    transposes back). K/V/Q projection MMs are interleaved into the
    ACT(exp)-bound attention stream via a deadline-tagged op queue.
"""
import os
import sys

import numpy as np

try:
    import ml_dtypes
except ImportError:  # pragma: no cover
    sys.path.insert(0, "/opt/trn_rl_repo")
    import ml_dtypes

for _p in ("/opt/trn_rl_repo",):
    if os.path.isdir(_p) and _p not in sys.path:
        sys.path.insert(0, _p)

try:
    import jax as _jax
    _jax.config.update("jax_compilation_cache_dir", "/tmp/jax_neff_cache")
    _jax.config.update("jax_persistent_cache_min_entry_size_bytes", -1)
    _jax.config.update("jax_persistent_cache_min_compile_time_secs", 0.0)
except Exception:
    pass

import concourse.bass as bass
import concourse.mybir as mybir
import concourse.tile as tile
from concourse import bacc
from concourse.bass_utils import run_bass_kernel_spmd
from concourse.masks import make_identity

B, T, C, H = 4, 4096, 1024, 128
P = 128           # partitions / tile edge
CK = C // P       # 8 contraction chunks
QW = 512          # q-block width
NQB = 4           # q-blocks per core (2048 queries)
KV_TILES = T // P # 32
NQ = NQB * QW
BF16 = ml_dtypes.bfloat16
SCALE = float(np.sqrt(H))

_prog_cache = {}


def _build_program(loop_n=None, loads_in_loop=True) -> bass.Bass:
    nc = bacc.Bacc("TRN2")
    dt = mybir.dt

    xT_d = nc.declare_dram_parameter("xT", [C, T], dt.bfloat16, isOutput=False)
    w_d = nc.declare_dram_parameter("w_all", [C, 3 * H], dt.bfloat16, isOutput=False)
    masks_d = nc.declare_dram_parameter("masks", [P, 4 * QW], dt.bfloat16, isOutput=False)
    rs_d = nc.declare_dram_parameter("rs", [P, 1], dt.float32, isOutput=False)
    eb_d = nc.declare_dram_parameter("eb", [P, 1], dt.float32, isOutput=False)
    out_d = nc.declare_dram_parameter("out", [H, NQ], dt.float32, isOutput=True)

    with tile.TileContext(nc) as tc:
        with (
            tc.tile_pool(name="consts", bufs=1) as consts,
            tc.tile_pool(name="bigx", bufs=1) as bigx,
            tc.tile_pool(name="persist", bufs=1) as persist,
            tc.tile_pool(name="psum_proj", bufs=1, space="PSUM") as psum_proj,
            tc.tile_pool(name="psum_tr", bufs=1, space="PSUM") as psum_tr,
            tc.tile_pool(name="psum_s", bufs=2, space="PSUM") as psum_s,
            tc.tile_pool(name="psum_o", bufs=1, space="PSUM") as psum_o,
            tc.tile_pool(name="psum_epi", bufs=1, space="PSUM") as psum_epi,
            tc.tile_pool(name="sb_p", bufs=6) as sb_p,
            tc.tile_pool(name="sb_u", bufs=2) as sb_u,
            tc.tile_pool(name="sb_vt", bufs=2) as sb_vt,
            tc.tile_pool(name="sb_o", bufs=2) as sb_o,
        ):
            f32, bf16 = dt.float32, dt.bfloat16
            import contextlib

            def loop_or_null(active):
                return tc.For_i(0, loop_n, 1) if (loop_n and active) else contextlib.nullcontext()

            with loop_or_null(loads_in_loop):

                # ---- constants ----
                w_sb = consts.tile([P, CK * 3 * H], bf16, tag="w")
                masks_sb = consts.tile([P, 4 * QW], bf16, tag="masks")
                rs_sb = consts.tile([P, 1], f32, tag="rs")
                eb_sb = consts.tile([P, 1], f32, tag="eb")
                ones_col = consts.tile([P, 1], f32, tag="onesc")
                nc.gpsimd.memset(ones_col[:], 1.0)
                ones_row = consts.tile([1, P], bf16, tag="onesr")
                nc.gpsimd.memset(ones_row[:], 1.0)
                ident_bf = consts.tile([P, P], bf16, tag="identb")
                make_identity(nc, ident_bf[:])

                def wq_s(ck):
                    return w_sb[:, ck * 3 * H: ck * 3 * H + H]

                def wk_s(ck):
                    return w_sb[:, ck * 3 * H + H: ck * 3 * H + 2 * H]

                def wv_s(ck):
                    return w_sb[:, ck * 3 * H + 2 * H: ck * 3 * H + 3 * H]

                # ---- stream inputs on the ACT hwdge queue (issue order =
                # consumption order); SP queue is reserved for the V xbar
                # transposes + output stores so they aren't head-of-line
                # blocked behind multi-MB input chunks ----
                x_sb = bigx.tile([P, CK * T], bf16, tag="x")
                x3 = x_sb[:].rearrange("p (ck t) -> p ck t", t=T)
                xd3 = xT_d.ap().rearrange("(ck p) t -> p ck t", p=P)

                nc.scalar.dma_start(
                    w_sb[:].rearrange("p (ck h) -> p ck h", h=3 * H),
                    w_d.ap().rearrange("(ck p) h -> p ck h", p=P),
                )
                nc.scalar.dma_start(x3[:, :, 0:QW], xd3[:, :, 0:QW])
                nc.scalar.dma_start(masks_sb[:], masks_d.ap()[:])
                nc.scalar.dma_start(rs_sb[:], rs_d.ap()[:])
                nc.scalar.dma_start(eb_sb[:], eb_d.ap()[:])
                for hseg in range(1, 8):
                    nc.scalar.dma_start(
                        x3[:, :, hseg * QW:(hseg + 1) * QW],
                        xd3[:, :, hseg * QW:(hseg + 1) * QW],
                    )

                kT_sb = persist.tile([P, T], bf16, tag="kT")
                v_sb = persist.tile([P, KV_TILES * H], bf16, tag="v")
                qT_sb = persist.tile([P, NQ], bf16, tag="qT")

                with loop_or_null(not loads_in_loop):
                    # ---- interleaved-op queue: (deadline_tile, x_seg, closure).
                    # Closures are popped either at their deadline (flush: last
                    # moment before the first attention trip that consumes their
                    # output — correctness) or at their statically scheduled
                    # attention pair (arrival-aware: never queue a PE op whose
                    # x segment hasn't landed, it would head-of-line block the
                    # in-order PE queue). ----
                    pending = []

                    def emit_qproj(m):
                        # q-block m reads program cols [1024m, 1024m+512)
                        st = {}

                        def mk(ck):
                            def op():
                                if ck == 0:
                                    st["ps"] = psum_proj.tile([P, QW], f32, tag="proj", name="psq")
                                nc.tensor.matmul(
                                    st["ps"][:],
                                    lhsT=wq_s(ck),
                                    rhs=x_sb[:, ck * T + 2 * m * QW: ck * T + 2 * m * QW + QW],
                                    start=(ck == 0), stop=(ck == CK - 1),
                                )
                            return op

                        tag, seg = 8 * m, 2 * m
                        for ck in range(CK):
                            pending.append((tag, seg, mk(ck)))
                        pending.append((tag, seg, lambda: nc.vector.tensor_scalar_mul(
                            qT_sb[:, m * QW:(m + 1) * QW], st["ps"][:], 1.0)))

                    def emit_projtile(t):
                        # K^T and V for kv tiles 4t..4t+3 (program cols 512t..+512)
                        tag, seg = 4 * t, t
                        stk, stv = {}, {}

                        def mkk(ck):
                            def op():
                                if ck == 0:
                                    stk["ps"] = psum_proj.tile([P, QW], f32, tag="proj", name="psk")
                                nc.tensor.matmul(
                                    stk["ps"][:],
                                    lhsT=wk_s(ck),
                                    rhs=x_sb[:, ck * T + t * QW: ck * T + (t + 1) * QW],
                                    start=(ck == 0), stop=(ck == CK - 1),
                                )
                            return op

                        def mkv(ck):
                            def op():
                                if ck == 0:
                                    stv["ps"] = psum_proj.tile([P, QW], f32, tag="proj", name="psv")
                                nc.tensor.matmul(
                                    stv["ps"][:],
                                    lhsT=wv_s(ck),
                                    rhs=x_sb[:, ck * T + t * QW: ck * T + (t + 1) * QW],
                                    start=(ck == 0), stop=(ck == CK - 1),
                                )
                            return op

                        for ck in range(CK):
                            pending.append((tag, seg, mkk(ck)))
                        pending.append((tag, seg, lambda: nc.vector.tensor_scalar_mul(
                            kT_sb[:, t * QW:(t + 1) * QW], stk["ps"][:], 1.0)))
                        for ck in range(CK):
                            pending.append((tag, seg, mkv(ck)))

                        def vcopy():
                            stv["vt"] = sb_vt.tile([P, QW], bf16, tag="vt", name="vt")
                            nc.vector.tensor_scalar_mul(stv["vt"][:], stv["ps"][:], 1.0)
                        pending.append((tag, seg, vcopy))

                        def mktr(s):
                            def op():
                                pt = psum_tr.tile([P, P], bf16, tag="tr", name="pt")
                                nc.tensor.transpose(
                                    pt[:], stv["vt"][:, s * P:(s + 1) * P], ident_bf[:]
                                )
                                nc.vector.tensor_scalar_mul(
                                    v_sb[:, (4 * t + s) * H:(4 * t + s + 1) * H],
                                    pt[:], 1.0)
                            return op

                        for s in range(4):
                            pending.append((4 * t + s, seg, mktr(s)))

                    # queue everything in deadline order
                    for m in range(NQB):
                        emit_qproj(m)
                        emit_projtile(2 * m)
                        emit_projtile(2 * m + 1)

                    # ---- static clock model: estimated start time (us) of each
                    # attention pair, assuming ~0.358 GB/us HBM and ACT-bound
                    # pairs of ~1.15us. Input order: w(0.75MB), seg0(1MB),
                    # masks(0.5MB), rs, seg1..7 (1MB each). ----
                    def arrive(seg):
                        mb = 0.75 + 1.0 + (0.53 + seg * 1.0 if seg >= 1 else 0.0)
                        return mb / 0.358

                    pair_start = {}
                    t_est = 0.0
                    for m in range(NQB):
                        for g in range(4 * (m + 1)):
                            t_est = max(
                                t_est,
                                arrive((2 * g + 1) // 4) + 0.4,
                                arrive(2 * m) + 0.3,
                            )
                            pair_start[(m, g)] = t_est
                            t_est += 1.15
                    # assign each closure to the first pair whose start covers
                    # its x segment arrival, capped at 6 closures per pair
                    all_pairs = sorted(pair_start, key=lambda k: pair_start[k])
                    cap = {p: 6 for p in all_pairs}
                    assign = {}
                    for idx, (tag, seg, _) in enumerate(pending):
                        tgt = None
                        for pkey in all_pairs:
                            if pair_start[pkey] >= arrive(seg) + 0.3 and cap[pkey] > 0:
                                tgt = pkey
                                break
                        if tgt is not None:
                            cap[tgt] -= 1
                            assign[idx] = tgt
                    pend_idx = list(range(len(pending)))

                    def pop_assigned(pkey):
                        morder = {k: i for i, k in enumerate(all_pairs)}
                        while pending:
                            idx = pend_idx[0]
                            tgt = assign.get(idx)
                            if tgt is not None and morder[tgt] <= morder[pkey]:
                                pend_idx.pop(0)
                                pending.pop(0)[2]()
                            else:
                                break

                    def flush(tile_id):
                        while pending and pending[0][0] <= tile_id:
                            pend_idx.pop(0)
                            pending.pop(0)[2]()

                    # ---- attention blocks ----
                    for m in range(NQB):
                        trips = 8 * (m + 1)
                        flush(8 * m)  # qT block m + early kv tiles
                        po = psum_o.tile([P, QW], f32, tag="po")
                        U0 = sb_u.tile([P, QW], f32, tag="U0")
                        U1 = sb_u.tile([P, QW], f32, tag="U1")
                        qs = qT_sb[:, m * QW:(m + 1) * QW]
                        for g in range(trips // 2):
                            flush(2 * g + 1)
                            s2 = psum_s.tile([P, 2 * QW], f32, tag="s")
                            for u in range(2):
                                j = 2 * g + u
                                nc.tensor.matmul(
                                    s2[:, u * QW:(u + 1) * QW],
                                    lhsT=kT_sb[:, j * P:(j + 1) * P],
                                    rhs=qs,
                                    start=True, stop=True,
                                )
                            p2 = sb_p.tile([P, 2 * QW], bf16, tag="p")
                            # pairs fully inside the role half-group window get
                            # the zeroing (r=0) via exp bias instead of a DVE mul
                            rolepair = 2 * g >= 8 * m + 4
                            nc.scalar.activation(
                                p2[:], s2[:], mybir.ActivationFunctionType.Exp,
                                bias=(eb_sb[:, 0:1] if rolepair else 0.0),
                            )
                            pop_assigned((m, g))
                            for u in range(2):
                                j = 2 * g + u
                                pj = p2[:, u * QW:(u + 1) * QW]
                                d = j - 8 * m
                                if 0 <= d < 4:
                                    nc.vector.tensor_mul(
                                        pj, pj, masks_sb[:, d * QW:(d + 1) * QW]
                                    )
                                ueng = nc.vector if u == 0 else nc.gpsimd
                                Ux = U0 if u == 0 else U1
                                if j < 2:
                                    ueng.tensor_copy(Ux[:], pj)
                                else:
                                    ueng.tensor_add(Ux[:], Ux[:], pj)
                                nc.tensor.matmul(
                                    po[:],
                                    lhsT=v_sb[:, j * H:(j + 1) * H],
                                    rhs=pj,
                                    start=(j == 0), stop=(j == trips - 1),
                                )
                        # epilogue: denom row, reciprocal, rank-1 broadcast, scale
                        epi = psum_epi.tile([P, QW], f32, tag="epi")
                        nc.tensor.matmul(
                            epi[0:1, :], lhsT=ones_col[:], rhs=U0[:],
                            start=True, stop=False,
                        )
                        nc.tensor.matmul(
                            epi[0:1, :], lhsT=ones_col[:], rhs=U1[:],
                            start=False, stop=True,
                        )
                        rec_row = sb_o.tile([1, QW], bf16, tag="rrow")
                        with nc.allow_low_precision(reason="bf16 softmax denom reciprocal; ~2^-9 rel, tol 2e-2"):
                            nc.vector.reciprocal(rec_row[:], epi[0:1, :])
                        nc.tensor.matmul(
                            epi[:], lhsT=ones_row[:], rhs=rec_row[:],
                            start=True, stop=True,
                        )
                        rec_b = sb_o.tile([P, QW], f32, tag="rb")
                        nc.vector.tensor_scalar_mul(rec_b[:], epi[:], 1.0)
                        o = sb_o.tile([P, QW], f32, tag="o")
                        nc.vector.tensor_mul(o[:], po[:], rec_b[:])
                        nc.sync.dma_start(
                            out_d.ap()[:, m * QW:(m + 1) * QW], o[:]
                        )
                    flush(KV_TILES)
    nc.compile()
    return nc


def _make_core_inputs(x, Wq, Wk, Wv):
    w_all = np.concatenate([Wq.T / SCALE, Wk.T, Wv.T], axis=1)  # [C, 3H]
    w_all = np.ascontiguousarray(w_all).astype(BF16)
    # static diagonal-window masks [P, 4*QW]: program tile d vs q-subtile s:
    # d == s -> tri (kv row p attends q col c iff p <= c); d > s -> 0; d < s -> 1
    tri = np.triu(np.ones((P, P), np.float32))
    masks = np.zeros((4, 4, P, P), np.float32)  # [d, s, p, c]
    for d in range(4):
        for s in range(4):
            if d == s:
                masks[d, s] = tri
            elif d < s:
                masks[d, s] = 1.0
    masks_flat = np.ascontiguousarray(
        masks.transpose(2, 0, 1, 3).reshape(P, 4 * QW)).astype(BF16)
    in_maps, qrows_all = [], []
    for c in range(8):
        b, r = c // 2, c % 2
        xT = np.ascontiguousarray(x[b].T)  # [C, T]
        if r:
            xT = np.ascontiguousarray(
                xT.reshape(C, 4, 2, QW)[:, :, ::-1, :].reshape(C, T))
        qrows = np.concatenate(
            [np.arange(QW * (r + 2 * m), QW * (r + 2 * m) + QW) for m in range(NQB)]
        )
        in_maps.append(dict(
            xT=xT.astype(BF16),
            w_all=w_all,
            masks=masks_flat,
            rs=np.full((P, 1), float(r), np.float32),
            eb=np.full((P, 1), 0.0 if r else -1e9, np.float32),
        ))
        qrows_all.append(qrows)
    return in_maps, qrows_all


def kernel(x, Wq, Wk, Wv):
    x = np.asarray(x, dtype=np.float32)
    if "nc" not in _prog_cache:
        _prog_cache["nc"] = _build_program()
    nc = _prog_cache["nc"]
    in_maps, qrows_all = _make_core_inputs(
        x, np.asarray(Wq, np.float32), np.asarray(Wk, np.float32),
        np.asarray(Wv, np.float32)
    )
    res = run_bass_kernel_spmd(nc, in_maps, list(range(8))).results
    full = np.zeros((B, T, H), np.float32)
    for c in range(8):
        full[c // 2][qrows_all[c]] = res[c]["out"].T
    return full


if __name__ == "__main__":
    nc = _build_program()
    print("program built ok")
